# revision 35
# baseline (speedup 1.0000x reference)
"""Causal self-attention (B=4, S=2048, D=1024, H=16, HD=64) on 8 trn2 cores.

Sharding: core c handles batch b = c//2 and head-group g = c%2 (8 heads).
Each core computes its 8 heads' attention plus the partial output
projection over its d-slice; the host adds the two partial y's per batch.

Device layout is fully transposed ([feature, seq]) so every matmul
contraction lands on the partition dim with no on-device transposes:
  qkvT = wqkvT^T @ xT        (bf16 inputs, fp32 psum, e on partitions)
  scoresT[s_k, s_q] = kT^T @ qT                (bf16, causal-trimmed)
  pT = exp(scoresT/8)        (ACT, bf16 out; triangular mask on boundary)
  out_aug[128, s_q] = v_aug^T @ pT   (bf16; rows 64-127 = ones block ->
                                      64 replicated softmax denominators)
  yT = wprojT^T @ (outT / denom)               (bf16 weights)

Schedule: the attention inner loop software-pipelines the score matmuls
one iteration ahead of PV so the ACT exp stream never starves, and the
pacer drops QKV(j+1)/proj filler chains into the exp-wait slot before
each PV. QKV weights are DMA'd in per-head-pair chunks (host-side layout
matches SBUF) so the first matmul starts as soon as chunk 0 + x(0) land.
All proj columns are deferred to the last attention column, where causal
attention leaves the PE starved while ACT (exp) is saturated. Softmax
denominators are inverted with the custom-DVE fast reciprocal staged
through SBUF (the custom op needs an unshifted SBUF source).
"""

from contextlib import ExitStack

import ml_dtypes
import numpy as np

import concourse.bacc as bacc
import concourse.mybir as mybir
import concourse.tile as tile
from concourse._compat import with_exitstack
from concourse.bass import ds, ts  # noqa: E402
from concourse.bass_utils import run_bass_kernel_spmd
from concourse.masks import make_upper_triangular

B, S, D = 4, 2048, 1024
H, HD = 16, 64
P = 128
GH = 8            # heads per core
DS = GH * HD      # 512, d-slice per core
E = 3 * DS        # 1536 qkv features per core
KD = D // P       # 8 contraction subtiles for qkv
KP = DS // P      # 4 contraction subtiles for proj
NJ = S // 512     # 4 s_q tiles of 512
NST = S // P      # 16 s_k tiles of 128
F32 = mybir.dt.float32
BF16 = mybir.dt.bfloat16
EXP = mybir.ActivationFunctionType.Exp


@with_exitstack
def _emit(ctx: ExitStack, tc: tile.TileContext, xT, wqk, wqv, wprojT, yT):
    nc = tc.nc

    xT_t = xT.rearrange("(ko ki) s -> ki ko s", ki=P)      # [128, 8, 2048]
    wqk_t = wqk.rearrange("p (m k f) -> p m k f", m=8, k=KD)   # [128,8,8,128]
    wqv_t = wqv.rearrange("p (k f) -> p k f", k=KD)        # [128, 8, 512]
    wp_t = wprojT.rearrange("(ko ki) e -> ki ko e", ki=P)  # [128, 4, 1024]
    yT_t = yT.rearrange("(mo mi) s -> mi mo s", mi=P)      # [128, 8, 2048]

    const = ctx.enter_context(tc.tile_pool(name="const", bufs=1))
    qk_pool = ctx.enter_context(tc.tile_pool(name="qkp", bufs=1))
    big = ctx.enter_context(tc.tile_pool(name="big", bufs=1))
    pt_pool = ctx.enter_context(tc.tile_pool(name="ptp", bufs=4))
    xin = ctx.enter_context(tc.tile_pool(name="xin", bufs=3))
    ot_pool = ctx.enter_context(tc.tile_pool(name="otp", bufs=4))
    sm = ctx.enter_context(tc.tile_pool(name="sm", bufs=2))
    yout = ctx.enter_context(tc.tile_pool(name="yo", bufs=3))
    ps_sc = ctx.enter_context(tc.tile_pool(name="ps_sc", bufs=3, space="PSUM"))
    ps_pv = ctx.enter_context(tc.tile_pool(name="ps_pv", bufs=1, space="PSUM"))

    xts = [None] * NJ

    def load_x(n):
        xt = xin.tile([P, KD, 512], BF16, tag="xt", name="xt")
        nc.sync.dma_start(xt[:], xT_t[:, :, ts(n, 512)])
        xts[n] = xt

    # x(0) and the first two Q/K weight chunks get the DMA bandwidth to
    # themselves; everything needed later (remaining chunks, V weights,
    # proj weights, x(1)) is gated behind a DVE-FIFO checkpoint (a tiny
    # memset on the dest tile) so it only starts streaming once the
    # prelude compute that precedes its consumer is underway. Without
    # this the DMA engines round-robin all ~6MB and every prelude chain
    # stalls ~1.4us on its weight chunk.
    load_x(0)
    wq = const.tile([P, 8, KD, P], BF16)
    for m in (0, 1, 2):
        nc.sync.dma_start(wq[:, m], wqk_t[:, m])
    wv = const.tile([P, KD, DS], BF16)
    wp = const.tile([P, KP, D], BF16)
    mask = const.tile([P, P], BF16)
    make_upper_triangular(nc, mask[:], val=1.0, diag=True)

    def gated_dma(dst, src):
        nc.vector.memset(dst[0:1, 0:1, 0:1], 0.0)
        nc.sync.dma_start(dst[:], src)

    # qkT: e-tiles 0-3 = q head pairs, 4-7 = k head pairs; [e_in, tile, s]
    qk = qk_pool.tile([P, 8, S], BF16)
    # v natural layout + 64-wide ones block per head: [s_in, s_tile, head, 128]
    # PV with this lhsT gives psum rows 0-63 = out, 64-127 = denom copies.
    vaug = big.tile([P, NST, GH, 2 * HD], BF16)
    ones = const.tile([P, 1], F32)

    def qkv_qk_chain(n, m):
        ps = ps_sc.tile([P, 1024], F32, tag="sc", name="ps")[:, 0:512]
        for k in range(KD):
            nc.tensor.matmul(ps[:], wq[:, m, k, :], xts[n][:, k, :],
                             start=(k == 0), stop=(k == KD - 1))
        nc.vector.tensor_copy(qk[:, m, ts(n, 512)], ps[:])

    def qkv_v_chain(n, ss):
        st = n * 4 + ss
        ps = ps_sc.tile([P, 1024], F32, tag="sc", name="ps")[:, 0:512]
        for k in range(KD):
            nc.tensor.matmul(ps[:], xts[n][:, k, ts(ss, P)], wv[:, k, :],
                             start=(k == 0), stop=(k == KD - 1))
        nc.vector.tensor_copy(vaug[:, st, :, 0:HD],
                              ps.rearrange("p (h d) -> p h d", h=GH))

    outTs = [None] * NJ

    def emit_sc(l, j, i):
        # both heads' score matmuls for s_k tile i (row-group packed K=64)
        t = i - 4 * j
        off = 128 * t if t > 0 else 0
        sc = ps_sc.tile([P, 1024], F32, tag="sc", name="sc")
        nc.tensor.matmul(sc[:, off:512], qk[0:64, 4 + l, ts(i, P)],
                         qk[0:64, l, ds(j * 512 + off, 512 - off)],
                         start=True, stop=True)
        nc.tensor.matmul(sc[:, 512 + off:1024], qk[64:128, 4 + l, ts(i, P)],
                         qk[64:128, l, ds(j * 512 + off, 512 - off)],
                         start=True, stop=True)
        return sc, off

    def normalize(l, j, pvs, last=False):
        # Per-head fast copies of the PV tiles to SBUF release each PSUM
        # bank as soon as possible for the next pair; softmax-normalize
        # then runs entirely in SBUF. (The custom-DVE recip needs an
        # unshifted SBUF source; the mult needs both SBUF operands on the
        # same base partition.) For the very last pair there is no
        # successor waiting on the banks: skip staging to shorten the
        # path into proj.
        outT = outTs[j]
        if last:
            srcs = pvs
        else:
            srcs = []
            for hh in (0, 1):
                st = sm.tile([P, 512], F32, tag=f"st{hh}", name="st")
                nc.vector.tensor_copy(st[:], pvs[hh][:])
                srcs.append(st)
        dt_ = sm.tile([HD, 1024], F32, tag="dt", name="dt")
        for hh in (0, 1):
            nc.vector.tensor_copy(dt_[:, ts(hh, 512)], srcs[hh][HD:2 * HD, :])
        rec = sm.tile([HD, 1024], F32, tag="rec", name="rec")
        nc.vector.reciprocal_approx_fast(rec[:], dt_[:])
        for hh in (0, 1):
            nc.vector.tensor_tensor(outT[hh * HD:(hh + 1) * HD, l, :],
                                    srcs[hh][0:HD, :],
                                    rec[:, ts(hh, 512)],
                                    mybir.AluOpType.mult)

    def attn_column(j, pacer):
        # flat unit stream over (pair l, s_k tile i); score matmuls are
        # emitted two units ahead of PV (3 PSUM buffers) so the ACT exp
        # stream never waits on a filler chain occupying the PE.
        imax = 4 * (j + 1)
        units = [(l, i) for l in range(4) for i in range(imax)]
        scq = [emit_sc(units[k][0], j, units[k][1]) for k in (0, 1)]
        if j > 0:
            pacer.kick()
        pv = None
        for k, (l, i) in enumerate(units):
            sc, off = scq.pop(0)
            t = i - 4 * j  # >=0 -> diagonal boundary tile
            scv = sc.rearrange("p (u f) -> p u f", u=2)
            pt = pt_pool.tile([P, 1024], BF16, tag="pt", name="pt")
            ptv = pt.rearrange("p (u f) -> p u f", u=2)
            nc.scalar.activation(ptv[:, :, off:512], scv[:, :, off:512],
                                 EXP, scale=0.125)
            if t >= 0:  # causal mask on the boundary 128-col block
                nc.vector.tensor_tensor(
                    ptv[:, :, off:off + P], ptv[:, :, off:off + P],
                    mask[:, None, :].to_broadcast((P, 2, P)),
                    mybir.AluOpType.mult)
            if k + 2 < len(units):
                scq.append(emit_sc(units[k + 2][0], j, units[k + 2][1]))
            pacer.tick()
            if i == 0:
                pv = [ps_pv.tile([P, 512], F32, tag="pva", name="pva"),
                      ps_pv.tile([P, 512], F32, tag="pvb", name="pvb")]
            nc.tensor.matmul(pv[0][:, off:512], vaug[:, i, 2 * l, :],
                             pt[:, off:512],
                             start=(i == 0), stop=(i == imax - 1))
            nc.tensor.matmul(pv[1][:, off:512], vaug[:, i, 2 * l + 1, :],
                             pt[:, 512 + off:1024],
                             start=(i == 0), stop=(i == imax - 1))
            if i == imax - 1:
                normalize(l, j, pv, last=(j == NJ - 1 and l == 3))

    def proj_col_chain(j, m):
        ps = ps_sc.tile([P, 1024], F32, tag="sc", name="ps")[:, 0:512]
        for k in range(KP):
            nc.tensor.matmul(ps[:], wp[:, k, ts(m, P)], outTs[j][:, k, :],
                             start=(k == 0), stop=(k == KP - 1))
        yt = yout.tile([P, 512], BF16, tag="yt", name="yt")
        nc.vector.tensor_copy(yt[:], ps[:])
        nc.sync.dma_start(yT_t[:, m, ts(j, 512)], yt[:])

    class Pacer:
        # Bresenham-paced emission of filler matmul chains between
        # attention iterations, to keep the PE dense (HAM stays warm).
        def __init__(self, thunks, total_ticks, lag=16):
            self.thunks = list(thunks)
            # lag > 0: keep thunks in reserve so flush() has filler to
            # cover the last pair's softmax-normalize window (last column).
            # lag < 0: lead slightly so the QKV(j+1) chains land before
            # column j+1's score prologue needs them.
            if lag > 0:
                self.total = max(1, total_ticks * (lag + 1) // lag)
            else:
                self.total = max(1, total_ticks * (-lag - 1) // -lag)
            self.ticks = 0
            self.fired = 0

        def tick(self):
            self.ticks += 1
            while (self.fired < len(self.thunks)
                   and self.fired * self.total < self.ticks * len(self.thunks)):
                self.thunks[self.fired]()
                self.fired += 1

        def kick(self):
            # force-fire one thunk so the PE has filler during the first
            # exp of a fresh column (Bresenham alone starts at tick ~3)
            if self.fired < len(self.thunks):
                self.thunks[self.fired]()
                self.fired += 1

        def flush(self):
            while self.fired < len(self.thunks):
                self.thunks[self.fired]()
                self.fired += 1

    # prelude: QKV for the first s-block. Each chain's qk-copy is a DVE
    # checkpoint; the weight chunk needed two chains later (and wv/wp/x1,
    # needed later still) starts streaming behind it.
    for m in range(8):
        qkv_qk_chain(0, m)
        if m < 5:
            gated_dma(wq[:, m + 3], wqk_t[:, m + 3])
        if m == 4:
            gated_dma(wv, wqv_t)
    for ss in range(4):
        qkv_v_chain(0, ss)
        if ss == 0:
            gated_dma(wp, wp_t)
    nc.vector.memset(ones[:], 1.0)
    nc.vector.tensor_copy(vaug[:, :, :, HD:], ones.to_broadcast((P, NST, GH, HD)))

    # attention column j; QKV(j+1) paced into columns 0-2, all proj
    # columns 0-2 paced into the attention-heavy last column.
    for j in range(NJ):
        outTs[j] = ot_pool.tile([P, KP, 512], BF16, tag="outT", name="outT")
        # prefetch x one column further ahead than its QKV consumers, so
        # a QKV(j+1) filler kicked at the top of column j never waits DMA
        if j == 0:
            load_x(1)
        if j + 2 < NJ:
            load_x(j + 2)
        thunks = []
        if j + 1 < NJ:
            for m in range(8):
                thunks.append(lambda n=j + 1, m=m: qkv_qk_chain(n, m))
            for ss in range(4):
                thunks.append(lambda n=j + 1, ss=ss: qkv_v_chain(n, ss))
        else:
            for jj in range(NJ - 1):
                for m in range(8):
                    thunks.append(lambda jj=jj, m=m: proj_col_chain(jj, m))
        pacer = Pacer(thunks, 4 * 4 * (j + 1),
                      lag=(6 if j == NJ - 1 else 16))
        attn_column(j, pacer)
        pacer.flush()
    for m in range(8):
        proj_col_chain(NJ - 1, m)


_NC = None


def build_nc():
    global _NC
    if _NC is not None:
        return _NC
    nc = bacc.Bacc("TRN2", target_bir_lowering=False, debug=False)
    xT = nc.dram_tensor("xT", [D, S], BF16, kind="ExternalInput")
    wqk = nc.dram_tensor("wqk", [P, 8 * KD * P], BF16, kind="ExternalInput")
    wqv = nc.dram_tensor("wqv", [P, KD * DS], BF16, kind="ExternalInput")
    wprojT = nc.dram_tensor("wprojT", [DS, D], BF16, kind="ExternalInput")
    yT = nc.dram_tensor("yT", [D, S], BF16, kind="ExternalOutput")
    with tile.TileContext(nc) as tc:
        _emit(tc, xT.ap(), wqk.ap(), wqv.ap(), wprojT.ap(), yT.ap())
    nc.compile()
    _NC = nc
    return nc


def make_in_maps(x, w_attn, w_proj):
    x = np.ascontiguousarray(np.asarray(x, dtype=np.float32))
    w_attn = np.asarray(w_attn, dtype=np.float32)
    w_proj = np.asarray(w_proj, dtype=np.float32)
    in_maps = []
    for c in range(8):
        b, g = divmod(c, 2)
        rows = slice(g * DS, (g + 1) * DS)
        wqk_c = np.concatenate(
            [w_attn[0 * D:1 * D][rows], w_attn[1 * D:2 * D][rows]],
            axis=0)                                       # [1024, 1024] (e, d)
        # SBUF layout [ki, m, ko, f]: element = wqk_c[m*128+f, ko*128+ki]
        wqk_l = wqk_c.reshape(8, P, KD, P).transpose(3, 0, 2, 1)
        wv_c = w_attn[2 * D:3 * D][rows]                  # [512, 1024] (e, d)
        # SBUF layout [ki, ko, f]: element = wv_c[f, ko*128+ki]
        wv_l = wv_c.reshape(DS, KD, P).transpose(2, 1, 0)
        in_maps.append({
            "xT": np.ascontiguousarray(x[b].T).astype(ml_dtypes.bfloat16),
            "wqk": np.ascontiguousarray(wqk_l.reshape(P, -1)
                                        ).astype(ml_dtypes.bfloat16),
            "wqv": np.ascontiguousarray(wv_l.reshape(P, -1)
                                        ).astype(ml_dtypes.bfloat16),
            "wprojT": np.ascontiguousarray(
                w_proj[:, rows].T).astype(ml_dtypes.bfloat16),  # [512, 1024]
        })
    return in_maps


def gather(results):
    y = np.empty((B, S, D), dtype=np.float32)
    for b in range(B):
        yT = (results[2 * b]["yT"].astype(np.float32)
              + results[2 * b + 1]["yT"].astype(np.float32))
        y[b] = yT.T
    return y


def run(x, w_attn, w_proj, trace=False, tmpdir=None):
    nc = build_nc()
    in_maps = make_in_maps(x, w_attn, w_proj)
    res = run_bass_kernel_spmd(nc, in_maps, list(range(8)),
                               trace=trace, tmpdir=tmpdir)
    return gather(res.results), res


def kernel(x, w_attn, w_proj):
    y, _ = run(x, w_attn, w_proj)
    return y
